# revision 4
# baseline (speedup 1.0000x reference)
"""Neural CDE Trainium2 kernel.

Data-parallel over batch: 8 cores x 64 batch. Per core, two independent
"chains" of 32 batch elements run the 255-step Euler scan concurrently so
engine work from one chain hides the serial-dependency latency of the other.

Per-chain layouts (b_loc in [0,32), h = 32*h_hi + h_lo, h_hi in [0,4)):
  z state   zT   [32, 128]  SBUF fp32, zT[h_lo, 32*h_hi + b_loc] = z[b, h]
  h1 = z@W1      [128, 32]  PSUM (f on partitions), 4 K=32 accumulating matmuls
  h = relu(h1+b1)[128, 32]  SBUF (ACT bias per-partition = b1)
  y = h@W2 + b2  [128, 384] PSUM, partitions p = b_loc + 32*h_hi,
                 free = (h_lo, c); built by 4 K=1 bias matmuls (ones x b2)
                 + 4 col-tiled K=128 matmuls (stationary h, moving W2 slice)
  t = tanh(y)    [128, 384] SBUF (ACT)
  u = t * dx     [128, 384] SBUF (DVE, dx broadcast over h_lo via 0-stride AP)
  zinc           [128, 32]  SBUF (DVE reduce over innermost c)
  zincT          [32, 128]  PSUM (PE transpose)
  zT_new = zT + zincT       (DVE)

z0, dX and the final readout/softmax are tiny (<0.01% of FLOPs) and run on
host numpy as part of sharding/gather.
"""

import numpy as np
from contextlib import ExitStack

B, T, C = 512, 256, 12
H, FF, O = 128, 128, 20
NCORES = 8
BLOC = B // NCORES          # 64 batch per core
NCHAIN = 2
BCH = BLOC // NCHAIN        # 32 batch per chain
NSTEP = T - 1               # 255
NG = 4                      # h groups of 32
DXCAP = None                # if set, dx buffers hold only DXCAP steps (timing experiments)
FD = BCH * C                # 384 free elems for y/t/u tiles

_CACHE = {}


def _build(nsteps=NSTEP):
    import concourse.bass as bass
    import concourse.mybir as mybir
    import concourse.tile as tile
    import concourse.bacc as bacc
    from concourse import masks

    f32 = mybir.dt.float32

    nc = bacc.Bacc("TRN2", target_bir_lowering=False, debug=False,
                   num_devices=NCORES)

    z0T_d = nc.dram_tensor("z0T", [NCHAIN, BCH, H], f32, kind="ExternalInput")
    W1_d = nc.dram_tensor("W1", [H, FF], f32, kind="ExternalInput")
    b1_d = nc.dram_tensor("b1", [FF, 1], f32, kind="ExternalInput")
    W2_d = nc.dram_tensor("W2", [FF, H * C], f32, kind="ExternalInput")
    b2_d = nc.dram_tensor("b2", [1, H * C], f32, kind="ExternalInput")
    nd = DXCAP or NSTEP
    dx_d = nc.dram_tensor("dxs", [NCHAIN, 128, nd * C], f32,
                          kind="ExternalInput")
    zout_d = nc.dram_tensor("zT_out", [NCHAIN, BCH, H], f32,
                            kind="ExternalOutput")

    with tile.TileContext(nc) as tc, ExitStack() as ctx:
        consts = ctx.enter_context(tc.tile_pool(name="consts", bufs=1))

        w1q = [consts.tile([32, FF], f32, tag=f"w1q{g}", name=f"w1q{g}") for g in range(NG)]
        for g in range(NG):
            nc.sync.dma_start(w1q[g][:], W1_d[32 * g:32 * (g + 1), :])
        b1c = consts.tile([FF, 1], f32, tag="b1c")
        nc.sync.dma_start(b1c[:], b1_d[:])
        w2 = consts.tile([FF, H * C], f32, tag="w2")
        nc.sync.dma_start(w2[:], W2_d[:])
        b2r = consts.tile([1, H * C], f32, tag="b2r")
        nc.sync.dma_start(b2r[:], b2_d[:])
        ones32 = consts.tile([1, 32], f32, tag="ones32")
        nc.vector.memset(ones32[:], 1.0)
        ident = consts.tile([128, 128], f32, tag="ident")
        masks.make_identity(nc, ident[:])
        dxs = [consts.tile([128, nd * C], f32, tag=f"dx{q}", name=f"dx{q}")
               for q in range(NCHAIN)]
        for q in range(NCHAIN):
            nc.sync.dma_start(dxs[q][:], dx_d[q])

        zt_p, h_p, t_p, u_p, zi_p, h1_p, y_p, ztp_p = [], [], [], [], [], [], [], []
        for q in range(NCHAIN):
            zt_p.append(ctx.enter_context(
                tc.tile_pool(name=f"zt{q}", bufs=2)))
            h_p.append(ctx.enter_context(
                tc.tile_pool(name=f"h{q}", bufs=2)))
            t_p.append(ctx.enter_context(
                tc.tile_pool(name=f"t{q}", bufs=2)))
            u_p.append(ctx.enter_context(
                tc.tile_pool(name=f"u{q}", bufs=2)))
            zi_p.append(ctx.enter_context(
                tc.tile_pool(name=f"zi{q}", bufs=2)))
            h1_p.append(ctx.enter_context(
                tc.tile_pool(name=f"h1ps{q}", bufs=1,
                             space=bass.MemorySpace.PSUM)))
            y_p.append(ctx.enter_context(
                tc.tile_pool(name=f"yps{q}", bufs=1,
                             space=bass.MemorySpace.PSUM)))
            ztp_p.append(ctx.enter_context(
                tc.tile_pool(name=f"ztps{q}", bufs=1,
                             space=bass.MemorySpace.PSUM)))

        # initial zT state
        zt = []
        for q in range(NCHAIN):
            z = zt_p[q].tile([BCH, H], f32, tag="zt", name=f"zt_init{q}")
            nc.sync.dma_start(z[:], z0T_d[q])
            zt.append(z)

        def step(q, k):
            # MM1: h1[f, b] = sum_h W1[h, f] * zT[h-layout]
            h1 = h1_p[q].tile([FF, BCH], f32, tag="h1")
            for g in range(NG):
                nc.tensor.matmul(h1[:], w1q[g][:],
                                 zt[q][:, 32 * g:32 * (g + 1)],
                                 start=(g == 0), stop=(g == NG - 1))
            # relu(h1 + b1)
            h = h_p[q].tile([FF, BCH], f32, tag="h")
            nc.scalar.activation(h[:], h1[:],
                                 mybir.ActivationFunctionType.Relu,
                                 bias=b1c[:])
            # y = b2 + h @ W2  (col-tiled into 4 partition groups)
            y = y_p[q].tile([128, FD], f32, tag="y")
            for g in range(NG):
                nc.tensor.matmul(y[32 * g:32 * (g + 1), :], ones32[:],
                                 b2r[:, FD * g:FD * (g + 1)],
                                 start=True, stop=False,
                                 tile_position=(0, 32 * g),
                                 skip_group_check=True)
                nc.tensor.matmul(y[32 * g:32 * (g + 1), :], h[:],
                                 w2[:, FD * g:FD * (g + 1)],
                                 start=False, stop=True,
                                 tile_position=(0, 32 * g),
                                 skip_group_check=True)
            # t = tanh(y)
            t = t_p[q].tile([128, FD], f32, tag="t")
            nc.scalar.activation(t[:], y[:],
                                 mybir.ActivationFunctionType.Tanh)
            # u = t * dx (dx broadcast over h_lo)
            u = u_p[q].tile([128, FD], f32, tag="u")
            kk = k % nd
            dxk = dxs[q][:, C * kk:C * (kk + 1)]
            nc.vector.tensor_tensor(
                u[:].rearrange("p (hl c) -> p hl c", c=C),
                t[:].rearrange("p (hl c) -> p hl c", c=C),
                dxk.unsqueeze(1).broadcast_to([128, 32, C]),
                mybir.AluOpType.mult)
            # zinc = sum_c u
            zi = zi_p[q].tile([128, 32], f32, tag="zi")
            nc.vector.reduce_sum(zi[:],
                                 u[:].rearrange("p (hl c) -> p hl c", c=C),
                                 axis=mybir.AxisListType.X)
            # transpose zinc and update zT
            zit = ztp_p[q].tile([BCH, H], f32, tag="zit")
            nc.tensor.transpose(zit[:], zi[:], ident[:])
            znew = zt_p[q].tile([BCH, H], f32, tag="zt")
            nc.vector.tensor_add(znew[:], zt[q][:], zit[:])
            zt[q] = znew

        for k in range(nsteps):
            for q in range(NCHAIN):
                step(q, k)

        for q in range(NCHAIN):
            nc.sync.dma_start(zout_d[q], zt[q][:])

    nc.compile()
    return nc


def _get_nc(nsteps=NSTEP):
    key = ("nc", nsteps)
    if key not in _CACHE:
        _CACHE[key] = _build(nsteps)
    return _CACHE[key]


def _prep_core(z0, dX, r):
    """Per-core input map. z0 [B, H] fp32, dX [B, T-1, C] fp32."""
    z0c = z0[BLOC * r:BLOC * (r + 1)]          # [64, 128]
    # [q, b_loc, h_hi, h_lo] -> [q, h_lo, h_hi, b_loc] -> [q, 32, 128]
    z0T = (z0c.reshape(NCHAIN, BCH, NG, 32)
              .transpose(0, 3, 2, 1)
              .reshape(NCHAIN, BCH, H)
              .astype(np.float32, copy=True))
    dxc = dX[BLOC * r:BLOC * (r + 1)]          # [64, 255, 12]
    nd = DXCAP or NSTEP
    dxq = np.empty((NCHAIN, 128, nd * C), np.float32)
    for q in range(NCHAIN):
        blk = dxc[BCH * q:BCH * (q + 1), :nd]      # [32, nd, 12]
        dxq[q] = np.tile(blk.reshape(BCH, nd * C), (NG, 1))
    return {"z0T": np.ascontiguousarray(z0T), "dxs": np.ascontiguousarray(dxq)}


def kernel(coeffs, times, W_init, b_init, W1, b1, W2, b2, W_out, b_out,
           _want_results=False):
    from concourse.bass_utils import run_bass_kernel_spmd

    coeffs = np.asarray(coeffs, np.float32)
    z0 = coeffs[:, 0] @ np.asarray(W_init, np.float32) + np.asarray(
        b_init, np.float32)                              # [B, H]
    dX = coeffs[:, 1:] - coeffs[:, :-1]                  # [B, T-1, C]

    shared = {
        "W1": np.ascontiguousarray(np.asarray(W1, np.float32)),
        "b1": np.ascontiguousarray(np.asarray(b1, np.float32).reshape(FF, 1)),
        "W2": np.ascontiguousarray(np.asarray(W2, np.float32)),
        "b2": np.ascontiguousarray(np.asarray(b2, np.float32).reshape(1, H * C)),
    }
    in_maps = [dict(shared, **_prep_core(z0, dX, r)) for r in range(NCORES)]

    nc = _get_nc()
    res = run_bass_kernel_spmd(nc, in_maps, core_ids=list(range(NCORES)))

    z_T = np.empty((B, H), np.float32)
    for r in range(NCORES):
        o = res.results[r]["zT_out"]                     # [2, 32, 128]
        # [q, h_lo, h_hi, b_loc] -> [q, b_loc, h_hi, h_lo] -> [64, 128]
        z_T[BLOC * r:BLOC * (r + 1)] = (
            o.reshape(NCHAIN, BCH, NG, 32).transpose(0, 3, 2, 1)
             .reshape(BLOC, H))
    logits = z_T @ np.asarray(W_out, np.float32) + np.asarray(
        b_out, np.float32)
    m = logits.max(axis=-1, keepdims=True)
    e = np.exp(logits - m)
    out = e / e.sum(axis=-1, keepdims=True)
    if _want_results:
        return out.astype(np.float32), res
    return out.astype(np.float32)



# revision 12
# speedup vs baseline: 1.5862x; 1.5862x over previous
"""Neural CDE Trainium2 kernel.

Data-parallel over batch: 8 cores x 64 batch. Per core, two independent
"chains" of 32 batch elements run the 255-step Euler scan concurrently so
engine work from one chain hides the serial-dependency latency of the other.

Per-chain layouts (b_loc in [0,32), h = 32*h_hi + h_lo, h_hi in [0,4)):
  z state   zT   [32, 128]  SBUF fp32, zT[h_lo, 32*h_hi + b_loc] = z[b, h]
  h1 = z@W1      [128, 32]  PSUM (f on partitions), 4 K=32 accumulating matmuls
  h = relu(h1+b1)[128, 32]  SBUF (ACT bias per-partition = b1)
  y = h@W2 + b2  [128, 384] PSUM, partitions p = b_loc + 32*h_hi,
                 free = (h_lo, c); built by 4 K=1 bias matmuls (ones x b2)
                 + 4 col-tiled K=128 matmuls (stationary h, moving W2 slice)
  t = tanh(y)    [128, 384] SBUF (ACT)
  u = t * dx     [128, 384] SBUF (DVE, dx broadcast over h_lo via 0-stride AP)
  zinc           [128, 32]  SBUF (DVE reduce over innermost c)
  zincT          [32, 128]  PSUM (PE transpose)
  zT_new = zT + zincT       (DVE)

z0, dX and the final readout/softmax are tiny (<0.01% of FLOPs) and run on
host numpy as part of sharding/gather.
"""

import numpy as np
from contextlib import ExitStack

B, T, C = 512, 256, 12
H, FF, O = 128, 128, 20
NCORES = 8
BLOC = B // NCORES          # 64 batch per core
NCHAIN = 2
BCH = BLOC // NCHAIN        # 32 batch per chain
NSTEP = T - 1               # 255
NG = 4                      # h groups of 32
DXCAP = None                # if set, dx buffers hold only DXCAP steps (timing experiments)
FD = BCH * C                # 384 free elems for y/t/u tiles

_CACHE = {}


def _build(nsteps=NSTEP):
    import concourse.bass as bass
    import concourse.mybir as mybir
    import concourse.tile as tile
    import concourse.bacc as bacc
    from concourse import masks

    f32 = mybir.dt.float32

    nc = bacc.Bacc("TRN2", target_bir_lowering=False, debug=False,
                   num_devices=NCORES)

    z0T_d = nc.dram_tensor("z0T", [NCHAIN, BCH, H], f32, kind="ExternalInput")
    W1_d = nc.dram_tensor("W1", [H, FF], f32, kind="ExternalInput")
    b1_d = nc.dram_tensor("b1", [FF, 1], f32, kind="ExternalInput")
    W2_d = nc.dram_tensor("W2", [FF, H * C], f32, kind="ExternalInput")
    b2_d = nc.dram_tensor("b2big", [128, FD], f32, kind="ExternalInput")
    nd = DXCAP or NSTEP
    dx_d = nc.dram_tensor("dxs", [NCHAIN, 128, nd * C], f32,
                          kind="ExternalInput")
    zout_d = nc.dram_tensor("zT_out", [NCHAIN, BCH, H], f32,
                            kind="ExternalOutput")

    with tile.TileContext(nc) as tc, ExitStack() as ctx:
        consts = ctx.enter_context(tc.tile_pool(name="consts", bufs=1))

        w1q = [consts.tile([32, FF], f32, tag=f"w1q{g}", name=f"w1q{g}") for g in range(NG)]
        for g in range(NG):
            nc.sync.dma_start(w1q[g][:], W1_d[32 * g:32 * (g + 1), :])
        b1c = consts.tile([FF, 1], f32, tag="b1c")
        nc.sync.dma_start(b1c[:], b1_d[:])
        w2 = consts.tile([FF, H * C], f32, tag="w2")
        nc.sync.dma_start(w2[:], W2_d[:])
        b2big = consts.tile([128, FD], f32, tag="b2big")
        nc.sync.dma_start(b2big[:], b2_d[:])
        ident = consts.tile([128, 128], f32, tag="ident")
        masks.make_identity(nc, ident[:])
        dxs = [consts.tile([128, nd * C], f32, tag=f"dx{q}", name=f"dx{q}")
               for q in range(NCHAIN)]
        for q in range(NCHAIN):
            nc.sync.dma_start(dxs[q][:], dx_d[q])

        zt_p, h_p, t_p, u_p, zi_p, h1_p, y_p, ztp_p = [], [], [], [], [], [], [], []
        yb_p = []
        for q in range(NCHAIN):
            yb_p.append(ctx.enter_context(
                tc.tile_pool(name=f"yb{q}", bufs=2)))
            zt_p.append(ctx.enter_context(
                tc.tile_pool(name=f"zt{q}", bufs=2)))
            h_p.append(ctx.enter_context(
                tc.tile_pool(name=f"h{q}", bufs=2)))
            t_p.append(ctx.enter_context(
                tc.tile_pool(name=f"t{q}", bufs=2)))
            u_p.append(ctx.enter_context(
                tc.tile_pool(name=f"u{q}", bufs=2)))
            zi_p.append(ctx.enter_context(
                tc.tile_pool(name=f"zi{q}", bufs=2)))
            h1_p.append(ctx.enter_context(
                tc.tile_pool(name=f"h1ps{q}", bufs=1,
                             space=bass.MemorySpace.PSUM)))
            y_p.append(ctx.enter_context(
                tc.tile_pool(name=f"yps{q}", bufs=1,
                             space=bass.MemorySpace.PSUM)))
            ztp_p.append(ctx.enter_context(
                tc.tile_pool(name=f"ztps{q}", bufs=1,
                             space=bass.MemorySpace.PSUM)))

        # initial zT state
        zt = []
        for q in range(NCHAIN):
            z = zt_p[q].tile([BCH, H], f32, tag="zt", name=f"zt_init{q}")
            nc.sync.dma_start(z[:], z0T_d[q])
            zt.append(z)

        def step(q, k):
            # MM1: h1[f, b] = sum_h W1[h, f] * zT[h-layout]
            h1 = h1_p[q].tile([FF, BCH], f32, tag="h1")
            for g in range(NG):
                nc.tensor.matmul(h1[:], w1q[g][:],
                                 zt[q][:, 32 * g:32 * (g + 1)],
                                 start=(g == 0), stop=(g == NG - 1))
            # relu(h1 + b1)
            h = h_p[q].tile([FF, BCH], f32, tag="h")
            nc.scalar.activation(h[:], h1[:],
                                 mybir.ActivationFunctionType.Relu,
                                 bias=b1c[:])
            # y = h @ W2  (col-tiled into 4 partition groups)
            y = y_p[q].tile([128, FD], f32, tag="y")
            for g in range(NG):
                nc.tensor.matmul(y[32 * g:32 * (g + 1), :], h[:],
                                 w2[:, FD * g:FD * (g + 1)],
                                 start=True, stop=True,
                                 tile_position=(0, 32 * g),
                                 skip_group_check=True)
            # yb = y + b2 (Pool), t = tanh(yb) (ACT)
            yb = yb_p[q].tile([128, FD], f32, tag="yb")
            nc.vector.tensor_tensor(yb[:], y[:], b2big[:], mybir.AluOpType.add)
            t = t_p[q].tile([128, FD], f32, tag="t")
            nc.scalar.activation(t[:], yb[:],
                                 mybir.ActivationFunctionType.Tanh)
            # u = t * dx (dx broadcast over h_lo)
            u = u_p[q].tile([128, FD], f32, tag="u")
            kk = k % nd
            dxk = dxs[q][:, C * kk:C * (kk + 1)]
            nc.gpsimd.tensor_tensor(
                u[:].rearrange("p (hl c) -> p hl c", c=C),
                t[:].rearrange("p (hl c) -> p hl c", c=C),
                dxk.unsqueeze(1).broadcast_to([128, 32, C]),
                mybir.AluOpType.mult)
            # zinc = sum_c u
            zi = zi_p[q].tile([128, 32], f32, tag="zi")
            nc.vector.reduce_sum(zi[:],
                                 u[:].rearrange("p (hl c) -> p hl c", c=C),
                                 axis=mybir.AxisListType.X)
            # transpose zinc and update zT
            zit = ztp_p[q].tile([BCH, H], f32, tag="zit")
            nc.tensor.transpose(zit[:], zi[:], ident[:])
            znew = zt_p[q].tile([BCH, H], f32, tag="zt")
            nc.vector.tensor_add(znew[:], zt[q][:], zit[:])
            zt[q] = znew

        for k in range(nsteps):
            for q in range(NCHAIN):
                step(q, k)

        for q in range(NCHAIN):
            nc.sync.dma_start(zout_d[q], zt[q][:])

    nc.compile()
    return nc


def _get_nc(nsteps=NSTEP):
    key = ("nc", nsteps)
    if key not in _CACHE:
        _CACHE[key] = _build(nsteps)
    return _CACHE[key]


def _prep_core(z0, dX, r):
    """Per-core input map. z0 [B, H] fp32, dX [B, T-1, C] fp32."""
    z0c = z0[BLOC * r:BLOC * (r + 1)]          # [64, 128]
    # [q, b_loc, h_hi, h_lo] -> [q, h_lo, h_hi, b_loc] -> [q, 32, 128]
    z0T = (z0c.reshape(NCHAIN, BCH, NG, 32)
              .transpose(0, 3, 2, 1)
              .reshape(NCHAIN, BCH, H)
              .astype(np.float32, copy=True))
    dxc = dX[BLOC * r:BLOC * (r + 1)]          # [64, 255, 12]
    nd = DXCAP or NSTEP
    dxq = np.empty((NCHAIN, 128, nd * C), np.float32)
    for q in range(NCHAIN):
        blk = dxc[BCH * q:BCH * (q + 1), :nd]      # [32, nd, 12]
        dxq[q] = np.tile(blk.reshape(BCH, nd * C), (NG, 1))
    return {"z0T": np.ascontiguousarray(z0T), "dxs": np.ascontiguousarray(dxq)}


def _shared_maps(W1, b1, W2, b2):
    """Per-core replicated weight tensors."""
    b2m = np.asarray(b2, np.float32).reshape(NG, 32, C)      # [h_hi, h_lo, c]
    # b2big[32*g + b, h_lo*C + c] = b2[32*g + h_lo, c], broadcast over b
    b2big = np.repeat(b2m.reshape(NG, 1, BCH * C), 32, axis=1).reshape(128, FD)
    return {
        "W1": np.ascontiguousarray(np.asarray(W1, np.float32)),
        "b1": np.ascontiguousarray(np.asarray(b1, np.float32).reshape(FF, 1)),
        "W2": np.ascontiguousarray(np.asarray(W2, np.float32)),
        "b2big": np.ascontiguousarray(b2big),
    }


def kernel(coeffs, times, W_init, b_init, W1, b1, W2, b2, W_out, b_out,
           _want_results=False):
    from concourse.bass_utils import run_bass_kernel_spmd

    coeffs = np.asarray(coeffs, np.float32)
    z0 = coeffs[:, 0] @ np.asarray(W_init, np.float32) + np.asarray(
        b_init, np.float32)                              # [B, H]
    dX = coeffs[:, 1:] - coeffs[:, :-1]                  # [B, T-1, C]

    shared = _shared_maps(W1, b1, W2, b2)
    in_maps = [dict(shared, **_prep_core(z0, dX, r)) for r in range(NCORES)]

    nc = _get_nc()
    res = run_bass_kernel_spmd(nc, in_maps, core_ids=list(range(NCORES)))

    z_T = np.empty((B, H), np.float32)
    for r in range(NCORES):
        o = res.results[r]["zT_out"]                     # [2, 32, 128]
        # [q, h_lo, h_hi, b_loc] -> [q, b_loc, h_hi, h_lo] -> [64, 128]
        z_T[BLOC * r:BLOC * (r + 1)] = (
            o.reshape(NCHAIN, BCH, NG, 32).transpose(0, 3, 2, 1)
             .reshape(BLOC, H))
    logits = z_T @ np.asarray(W_out, np.float32) + np.asarray(
        b_out, np.float32)
    m = logits.max(axis=-1, keepdims=True)
    e = np.exp(logits - m)
    out = e / e.sum(axis=-1, keepdims=True)
    if _want_results:
        return out.astype(np.float32), res
    return out.astype(np.float32)



# revision 13
# speedup vs baseline: 2.6795x; 1.6893x over previous
"""Neural CDE Trainium2 kernel.

Data-parallel over batch: 8 cores x 64 batch. Per core, two independent
"chains" of 32 batch elements run the 255-step Euler scan concurrently so
engine work from one chain hides the serial-dependency latency of the other.

Per-chain layouts (b in [0,32), h = 32*g + h_lo, g in [0,4)):
  z state   zT  [128, 32]  SBUF fp32, zT[h, b] = z[b, h]
  h1 = W1^T zT  [128, 32]  PSUM, one K=128 matmul (stationary W1 [h, f])
  h = relu(h1+b1)[128, 32] SBUF (ACT bias per-partition = b1)
  y = h @ W2    [128, 384] PSUM, partitions p = 32*g + b, free = (h_lo, c);
                4 col-tiled concurrent K=128 matmuls (stationary h at
                tile_position (0, 32g), moving W2 chunk g)
  yb = y + b2   [128, 384] SBUF (DVE; b2big pre-broadcast per partition grp)
  t = tanh(yb)  [128, 384] SBUF (ACT)
  u = t * dx    [128, 384] SBUF (Pool, dx broadcast over h_lo via 0-stride)
  zi            [128, 32]  SBUF (DVE reduce over innermost c) = [(g,b), h_lo]
  zit           [128, 32]  SBUF (DVE 32x32 block transpose) = [h, b]
  zT_new = zT + zit        (DVE)

z0, dX and the final readout/softmax are tiny (<0.01% of FLOPs) and run on
host numpy as part of sharding/gather.
"""

import numpy as np
from contextlib import ExitStack

B, T, C = 512, 256, 12
H, FF, O = 128, 128, 20
NCORES = 8
BLOC = B // NCORES          # 64 batch per core
NCHAIN = 2
BCH = BLOC // NCHAIN        # 32 batch per chain
NSTEP = T - 1               # 255
NG = 4                      # h groups of 32
FD = BCH * C                # 384 free elems for y/t/u tiles

_CACHE = {}


def _build(nsteps=NSTEP):
    import concourse.bass as bass
    import concourse.mybir as mybir
    import concourse.tile as tile
    import concourse.bacc as bacc

    f32 = mybir.dt.float32

    nc = bacc.Bacc("TRN2", target_bir_lowering=False, debug=False,
                   num_devices=NCORES)

    z0T_d = nc.dram_tensor("z0T", [NCHAIN, H, BCH], f32, kind="ExternalInput")
    W1_d = nc.dram_tensor("W1", [H, FF], f32, kind="ExternalInput")
    b1_d = nc.dram_tensor("b1", [FF, 1], f32, kind="ExternalInput")
    W2_d = nc.dram_tensor("W2", [FF, H * C], f32, kind="ExternalInput")
    b2_d = nc.dram_tensor("b2big", [128, FD], f32, kind="ExternalInput")
    nd = NSTEP
    dx_d = nc.dram_tensor("dxs", [NCHAIN, 128, nd * C], f32,
                          kind="ExternalInput")
    zout_d = nc.dram_tensor("zT_out", [NCHAIN, H, BCH], f32,
                            kind="ExternalOutput")

    with tile.TileContext(nc) as tc, ExitStack() as ctx:
        consts = ctx.enter_context(tc.tile_pool(name="consts", bufs=1))

        w1 = consts.tile([H, FF], f32, tag="w1")
        nc.sync.dma_start(w1[:], W1_d[:])
        b1c = consts.tile([FF, 1], f32, tag="b1c")
        nc.sync.dma_start(b1c[:], b1_d[:])
        w2 = consts.tile([FF, H * C], f32, tag="w2")
        nc.sync.dma_start(w2[:], W2_d[:])
        b2big = consts.tile([128, FD], f32, tag="b2big")
        nc.sync.dma_start(b2big[:], b2_d[:])
        dxs = [consts.tile([128, nd * C], f32, tag=f"dx{q}", name=f"dx{q}")
               for q in range(NCHAIN)]
        for q in range(NCHAIN):
            nc.sync.dma_start(dxs[q][:], dx_d[q])

        pools = {}
        for q in range(NCHAIN):
            pools[q] = {
                "zt": ctx.enter_context(tc.tile_pool(name=f"zt{q}", bufs=2)),
                "h": ctx.enter_context(tc.tile_pool(name=f"h{q}", bufs=2)),
                "yb": ctx.enter_context(tc.tile_pool(name=f"yb{q}", bufs=2)),
                "t": ctx.enter_context(tc.tile_pool(name=f"t{q}", bufs=2)),
                "u": ctx.enter_context(tc.tile_pool(name=f"u{q}", bufs=2)),
                "zi": ctx.enter_context(tc.tile_pool(name=f"zi{q}", bufs=2)),
                "zit": ctx.enter_context(tc.tile_pool(name=f"zit{q}", bufs=2)),
                "h1": ctx.enter_context(tc.tile_pool(
                    name=f"h1ps{q}", bufs=2, space=bass.MemorySpace.PSUM)),
                "y": ctx.enter_context(tc.tile_pool(
                    name=f"yps{q}", bufs=2, space=bass.MemorySpace.PSUM)),
            }

        # initial zT state
        zt = []
        for q in range(NCHAIN):
            z = pools[q]["zt"].tile([H, BCH], f32, tag="zt", name=f"zt_init{q}")
            nc.sync.dma_start(z[:], z0T_d[q])
            zt.append(z)

        def step(q, k):
            p = pools[q]
            # MM1: h1[f, b] = sum_h W1[h, f] * zT[h, b]
            h1 = p["h1"].tile([FF, BCH], f32, tag="h1")
            nc.tensor.matmul(h1[:], w1[:], zt[q][:], start=True, stop=True)
            # h = relu(h1 + b1)
            h = p["h"].tile([FF, BCH], f32, tag="h")
            nc.scalar.activation(h[:], h1[:],
                                 mybir.ActivationFunctionType.Relu,
                                 bias=b1c[:])
            # y = h @ W2 (col-tiled into 4 concurrent partition groups)
            y = p["y"].tile([128, FD], f32, tag="y")
            for g in range(NG):
                nc.tensor.matmul(y[32 * g:32 * (g + 1), :], h[:],
                                 w2[:, FD * g:FD * (g + 1)],
                                 start=True, stop=True,
                                 tile_position=(0, 32 * g),
                                 skip_group_check=True)
            # yb = y + b2 (DVE reads PSUM), t = tanh(yb) (ACT)
            yb = p["yb"].tile([128, FD], f32, tag="yb")
            nc.vector.tensor_tensor(yb[:], y[:], b2big[:], mybir.AluOpType.add)
            t = p["t"].tile([128, FD], f32, tag="t")
            nc.scalar.activation(t[:], yb[:],
                                 mybir.ActivationFunctionType.Tanh)
            # u = t * dx (dx broadcast over h_lo) on Pool
            u = p["u"].tile([128, FD], f32, tag="u")
            kk = k % nd
            dxk = dxs[q][:, C * kk:C * (kk + 1)]
            nc.gpsimd.tensor_tensor(
                u[:].rearrange("p (hl c) -> p hl c", c=C),
                t[:].rearrange("p (hl c) -> p hl c", c=C),
                dxk.unsqueeze(1).broadcast_to([128, BCH, C]),
                mybir.AluOpType.mult)
            # zi[(g,b), h_lo] = sum_c u
            zi = p["zi"].tile([128, BCH], f32, tag="zi")
            nc.vector.reduce_sum(zi[:],
                                 u[:].rearrange("p (hl c) -> p hl c", c=C),
                                 axis=mybir.AxisListType.X)
            # block-transpose to zit[h, b] and update zT
            zit = p["zit"].tile([128, BCH], f32, tag="zit")
            nc.vector.transpose(zit[:], zi[:])
            znew = p["zt"].tile([H, BCH], f32, tag="zt")
            nc.vector.tensor_add(znew[:], zt[q][:], zit[:])
            zt[q] = znew

        for k in range(nsteps):
            for q in range(NCHAIN):
                step(q, k)

        for q in range(NCHAIN):
            nc.sync.dma_start(zout_d[q], zt[q][:])

    nc.compile()
    return nc


def _get_nc(nsteps=NSTEP):
    key = ("nc", nsteps)
    if key not in _CACHE:
        _CACHE[key] = _build(nsteps)
    return _CACHE[key]


def _prep_core(z0, dX, r):
    """Per-core input map. z0 [B, H] fp32, dX [B, T-1, C] fp32."""
    z0c = z0[BLOC * r:BLOC * (r + 1)]          # [64, 128]
    # [q, b, h] -> [q, h, b]
    z0T = (z0c.reshape(NCHAIN, BCH, H)
              .transpose(0, 2, 1)
              .astype(np.float32, copy=True))
    dxc = dX[BLOC * r:BLOC * (r + 1)]          # [64, 255, 12]
    nd = NSTEP
    dxq = np.empty((NCHAIN, 128, nd * C), np.float32)
    for q in range(NCHAIN):
        blk = dxc[BCH * q:BCH * (q + 1), :nd]      # [32, nd, 12]
        dxq[q] = np.tile(blk.reshape(BCH, nd * C), (NG, 1))
    return {"z0T": np.ascontiguousarray(z0T), "dxs": np.ascontiguousarray(dxq)}


def _shared_maps(W1, b1, W2, b2):
    """Per-core replicated weight tensors."""
    b2m = np.asarray(b2, np.float32).reshape(NG, 32, C)      # [g, h_lo, c]
    # b2big[32*g + b, h_lo*C + c] = b2[32*g + h_lo, c], broadcast over b
    b2big = np.repeat(b2m.reshape(NG, 1, BCH * C), 32, axis=1).reshape(128, FD)
    return {
        "W1": np.ascontiguousarray(np.asarray(W1, np.float32)),
        "b1": np.ascontiguousarray(np.asarray(b1, np.float32).reshape(FF, 1)),
        "W2": np.ascontiguousarray(np.asarray(W2, np.float32)),
        "b2big": np.ascontiguousarray(b2big),
    }


def kernel(coeffs, times, W_init, b_init, W1, b1, W2, b2, W_out, b_out,
           _want_results=False):
    from concourse.bass_utils import run_bass_kernel_spmd

    coeffs = np.asarray(coeffs, np.float32)
    z0 = coeffs[:, 0] @ np.asarray(W_init, np.float32) + np.asarray(
        b_init, np.float32)                              # [B, H]
    dX = coeffs[:, 1:] - coeffs[:, :-1]                  # [B, T-1, C]

    shared = _shared_maps(W1, b1, W2, b2)
    in_maps = [dict(shared, **_prep_core(z0, dX, r)) for r in range(NCORES)]

    nc = _get_nc()
    res = run_bass_kernel_spmd(nc, in_maps, core_ids=list(range(NCORES)))

    z_T = np.empty((B, H), np.float32)
    for r in range(NCORES):
        o = res.results[r]["zT_out"]                     # [q, H, BCH]
        z_T[BLOC * r:BLOC * (r + 1)] = (
            o.transpose(0, 2, 1).reshape(BLOC, H))
    logits = z_T @ np.asarray(W_out, np.float32) + np.asarray(
        b_out, np.float32)
    m = logits.max(axis=-1, keepdims=True)
    e = np.exp(logits - m)
    out = e / e.sum(axis=-1, keepdims=True)
    if _want_results:
        return out.astype(np.float32), res
    return out.astype(np.float32)


# revision 14
# speedup vs baseline: 2.8148x; 1.0505x over previous
"""Neural CDE Trainium2 kernel.

Data-parallel over batch: 8 cores x 64 batch. Per core, two independent
"chains" of 32 batch elements run the 255-step Euler scan concurrently so
engine work from one chain hides the serial-dependency latency of the other.

Critical-cycle-oriented design (wall time ~= per-chain dependency cycle):
  state s[f, b] = W1^T z + b1 (SBUF), updated incrementally via the
  linearity of MM1:  s_{k+1} = s_k + W1^T zinc_k.  The z state itself is
  accumulated OFF the critical path on Pool; only s is on the cycle.

Per-chain layouts (b in [0,32), h = 32*g + h_lo, g in [0,4)):
  ds = W1^T zit [128, 32]  PSUM, one K=128 matmul (stationary W1 [h, f])
  s  = s + ds   [128, 32]  SBUF (DVE)
  h = relu(s)   [128, 32]  SBUF (DVE tensor_scalar max 0)
  y = b2 + h@W2 [128, 384] PSUM, partitions p = 32*g + b, free = (h_lo, c);
                per group g: K=1 bias matmul (ones x b2 chunk, start) early
                + K=128 main matmul (stop) - 4 groups run concurrently via
                col tiling at tile_position (0, 32g)
  t = tanh(y)   [128, 384] SBUF (ACT reads PSUM)
  u = t * dx    [128, 384] SBUF (DVE, dx broadcast over h_lo via 0-stride)
  zi            [128, 32]  SBUF (DVE reduce over innermost c) = [(g,b), h_lo]
  zit           [128, 32]  SBUF (DVE 32x32 block transpose) = [h, b] = zinc
  z_new = z + zit          (Pool, off critical path; z only needed at end)

z0, dX and the final readout/softmax are tiny (<0.01% of FLOPs) and run on
host numpy as part of sharding/gather.
"""

import numpy as np
from contextlib import ExitStack

B, T, C = 512, 256, 12
H, FF, O = 128, 128, 20
NCORES = 8
BLOC = B // NCORES          # 64 batch per core
NCHAIN = 2
BCH = BLOC // NCHAIN        # 32 batch per chain
NSTEP = T - 1               # 255
NG = 4                      # h groups of 32
FD = BCH * C                # 384 free elems for y/t/u tiles

_CACHE = {}


def _build(nsteps=NSTEP):
    import concourse.bass as bass
    import concourse.mybir as mybir
    import concourse.tile as tile
    import concourse.bacc as bacc

    f32 = mybir.dt.float32

    nc = bacc.Bacc("TRN2", target_bir_lowering=False, debug=False,
                   num_devices=NCORES)

    z0T_d = nc.dram_tensor("z0T", [NCHAIN, H, BCH], f32, kind="ExternalInput")
    W1_d = nc.dram_tensor("W1", [H, FF], f32, kind="ExternalInput")
    b1_d = nc.dram_tensor("b1", [FF, 1], f32, kind="ExternalInput")
    W2_d = nc.dram_tensor("W2", [FF, H * C], f32, kind="ExternalInput")
    b2_d = nc.dram_tensor("b2row", [1, H * C], f32, kind="ExternalInput")
    nd = NSTEP
    dx_d = nc.dram_tensor("dxs", [NCHAIN, 128, nd * C], f32,
                          kind="ExternalInput")
    zout_d = nc.dram_tensor("zT_out", [NCHAIN, H, BCH], f32,
                            kind="ExternalOutput")

    with tile.TileContext(nc) as tc, ExitStack() as ctx:
        consts = ctx.enter_context(tc.tile_pool(name="consts", bufs=1))

        w1 = consts.tile([H, FF], f32, tag="w1")
        nc.sync.dma_start(w1[:], W1_d[:])
        b1c = consts.tile([FF, 1], f32, tag="b1c")
        nc.sync.dma_start(b1c[:], b1_d[:])
        w2 = consts.tile([FF, H * C], f32, tag="w2")
        nc.sync.dma_start(w2[:], W2_d[:])
        b2row = consts.tile([1, H * C], f32, tag="b2row")
        nc.sync.dma_start(b2row[:], b2_d[:])
        ones32 = consts.tile([1, 32], f32, tag="ones32")
        nc.vector.memset(ones32[:], 1.0)
        dxs = [consts.tile([128, nd * C], f32, tag=f"dx{q}", name=f"dx{q}")
               for q in range(NCHAIN)]
        for q in range(NCHAIN):
            nc.sync.dma_start(dxs[q][:], dx_d[q])

        pools = {}
        for q in range(NCHAIN):
            pools[q] = {
                "zt": ctx.enter_context(tc.tile_pool(name=f"zt{q}", bufs=2)),
                "s": ctx.enter_context(tc.tile_pool(name=f"s{q}", bufs=2)),
                "h": ctx.enter_context(tc.tile_pool(name=f"h{q}", bufs=2)),
                "t": ctx.enter_context(tc.tile_pool(name=f"t{q}", bufs=2)),
                "u": ctx.enter_context(tc.tile_pool(name=f"u{q}", bufs=2)),
                "zi": ctx.enter_context(tc.tile_pool(name=f"zi{q}", bufs=2)),
                "zit": ctx.enter_context(tc.tile_pool(name=f"zit{q}", bufs=2)),
                "ds": ctx.enter_context(tc.tile_pool(
                    name=f"dsps{q}", bufs=2, space=bass.MemorySpace.PSUM)),
                "y": ctx.enter_context(tc.tile_pool(
                    name=f"yps{q}", bufs=2, space=bass.MemorySpace.PSUM)),
            }

        # initial z state
        zt, s_st = [], [None] * NCHAIN
        for q in range(NCHAIN):
            z = pools[q]["zt"].tile([H, BCH], f32, tag="zt", name=f"zt_init{q}")
            nc.sync.dma_start(z[:], z0T_d[q])
            zt.append(z)

        def step(q, k):
            p = pools[q]
            # bias seed: y[32g.., :] = b2 chunk (no data deps; runs early)
            y = p["y"].tile([128, FD], f32, tag="y")
            for g in range(NG):
                nc.tensor.matmul(y[32 * g:32 * (g + 1), :], ones32[:],
                                 b2row[:, FD * g:FD * (g + 1)],
                                 start=True, stop=False,
                                 tile_position=(0, 32 * g),
                                 skip_group_check=True)
            # MM1 delta: ds[f, b] = W1^T @ (zit if k else z0)
            src = zt[q] if k == 0 else step.zit_prev[q]
            ds = p["ds"].tile([FF, BCH], f32, tag="ds")
            nc.tensor.matmul(ds[:], w1[:], src[:], start=True, stop=True)
            # s update + relu (DVE, back to back)
            snew = p["s"].tile([FF, BCH], f32, tag="s")
            if k == 0:
                nc.vector.tensor_scalar(snew[:], ds[:], b1c[:], None,
                                        mybir.AluOpType.add)
            else:
                nc.vector.tensor_add(snew[:], s_st[q][:], ds[:])
            h = p["h"].tile([FF, BCH], f32, tag="h")
            nc.vector.tensor_scalar(h[:], snew[:], 0.0, None,
                                    mybir.AluOpType.max)
            s_st[q] = snew
            # main MM2, accumulating onto bias seed (4 concurrent col groups)
            for g in range(NG):
                nc.tensor.matmul(y[32 * g:32 * (g + 1), :], h[:],
                                 w2[:, FD * g:FD * (g + 1)],
                                 start=False, stop=True,
                                 tile_position=(0, 32 * g),
                                 skip_group_check=True)
            # t = tanh(y) (ACT reads PSUM)
            t = p["t"].tile([128, FD], f32, tag="t")
            nc.scalar.activation(t[:], y[:],
                                 mybir.ActivationFunctionType.Tanh)
            # u = t * dx; zi = sum_c u; zit = blocktranspose(zi)  (all DVE)
            u = p["u"].tile([128, FD], f32, tag="u")
            kk = k % nd
            dxk = dxs[q][:, C * kk:C * (kk + 1)]
            nc.vector.tensor_tensor(
                u[:].rearrange("p (hl c) -> p hl c", c=C),
                t[:].rearrange("p (hl c) -> p hl c", c=C),
                dxk.unsqueeze(1).broadcast_to([128, BCH, C]),
                mybir.AluOpType.mult)
            zi = p["zi"].tile([128, BCH], f32, tag="zi")
            nc.vector.reduce_sum(zi[:],
                                 u[:].rearrange("p (hl c) -> p hl c", c=C),
                                 axis=mybir.AxisListType.X)
            zit = p["zit"].tile([128, BCH], f32, tag="zit")
            nc.vector.transpose(zit[:], zi[:])
            step.zit_prev[q] = zit
            # z accumulation off the critical path (Pool)
            znew = p["zt"].tile([H, BCH], f32, tag="zt")
            nc.gpsimd.tensor_add(znew[:], zt[q][:], zit[:])
            zt[q] = znew

        step.zit_prev = [None] * NCHAIN

        for k in range(nsteps):
            for q in range(NCHAIN):
                step(q, k)

        for q in range(NCHAIN):
            nc.sync.dma_start(zout_d[q], zt[q][:])

    nc.compile()
    return nc


def _get_nc(nsteps=NSTEP):
    key = ("nc", nsteps)
    if key not in _CACHE:
        _CACHE[key] = _build(nsteps)
    return _CACHE[key]


def _prep_core(z0, dX, r):
    """Per-core input map. z0 [B, H] fp32, dX [B, T-1, C] fp32."""
    z0c = z0[BLOC * r:BLOC * (r + 1)]          # [64, 128]
    # [q, b, h] -> [q, h, b]
    z0T = (z0c.reshape(NCHAIN, BCH, H)
              .transpose(0, 2, 1)
              .astype(np.float32, copy=True))
    dxc = dX[BLOC * r:BLOC * (r + 1)]          # [64, 255, 12]
    nd = NSTEP
    dxq = np.empty((NCHAIN, 128, nd * C), np.float32)
    for q in range(NCHAIN):
        blk = dxc[BCH * q:BCH * (q + 1), :nd]      # [32, nd, 12]
        dxq[q] = np.tile(blk.reshape(BCH, nd * C), (NG, 1))
    return {"z0T": np.ascontiguousarray(z0T), "dxs": np.ascontiguousarray(dxq)}


def _shared_maps(W1, b1, W2, b2):
    """Per-core replicated weight tensors."""
    return {
        "W1": np.ascontiguousarray(np.asarray(W1, np.float32)),
        "b1": np.ascontiguousarray(np.asarray(b1, np.float32).reshape(FF, 1)),
        "W2": np.ascontiguousarray(np.asarray(W2, np.float32)),
        "b2row": np.ascontiguousarray(np.asarray(b2, np.float32).reshape(1, H * C)),
    }


def kernel(coeffs, times, W_init, b_init, W1, b1, W2, b2, W_out, b_out,
           _want_results=False):
    from concourse.bass_utils import run_bass_kernel_spmd

    coeffs = np.asarray(coeffs, np.float32)
    z0 = coeffs[:, 0] @ np.asarray(W_init, np.float32) + np.asarray(
        b_init, np.float32)                              # [B, H]
    dX = coeffs[:, 1:] - coeffs[:, :-1]                  # [B, T-1, C]

    shared = _shared_maps(W1, b1, W2, b2)
    in_maps = [dict(shared, **_prep_core(z0, dX, r)) for r in range(NCORES)]

    nc = _get_nc()
    res = run_bass_kernel_spmd(nc, in_maps, core_ids=list(range(NCORES)))

    z_T = np.empty((B, H), np.float32)
    for r in range(NCORES):
        o = res.results[r]["zT_out"]                     # [q, H, BCH]
        z_T[BLOC * r:BLOC * (r + 1)] = (
            o.transpose(0, 2, 1).reshape(BLOC, H))
    logits = z_T @ np.asarray(W_out, np.float32) + np.asarray(
        b_out, np.float32)
    m = logits.max(axis=-1, keepdims=True)
    e = np.exp(logits - m)
    out = e / e.sum(axis=-1, keepdims=True)
    if _want_results:
        return out.astype(np.float32), res
    return out.astype(np.float32)
